# revision 19
# baseline (speedup 1.0000x reference)
"""EpsSupInfoNCE loss on 8 Trainium2 NeuronCores.

Math: logits l_ij = (e_i . e_j)/temp, same_ij = (label_i == label_j).
  S_j  = sum_i exp(l_ij) * (1 - same_ij)          (masked column sums)
  ce_ij = log(exp(l_ij - eps) + S_j) - l_ij       (same-label i != j)
       = log S_j - l_ij + log1p(exp(l_ij - eps)/S_j)
  loss = sum_j sum_i ce_ij / count_j / B

Only S_j needs the full B x B pass; the numerator has ~B^2/NCLS terms
and is computed EXACTLY on the host (f64 class-block gemms) from the
chip's S_j.  So the chip computes just the masked exp column sums:

- Columns are label-sorted and sharded over 8 cores (1024 each).  Each
  core's rows are host-permuted so all of its same-label rows land in
  row-block 0 (span 1024+2*maxclass <= 2048); blocks 1-3 are mask-free,
  which keeps the DMA footprint down (one-hot plane only for block 0).
- Block 0 fuses logits+mask in one fp8e4m3 DoubleRow matmul:
  K=256 = [embeds(128) ; onehot(128)] as two 128-partition k-subtiles
  with (-4.5*onehot) on the lhs side, so same-label pairs get
  l - 4.5/temp (exp -> ~e-50 = masked).  Blocks 1-3: plain fp8 K=128.
- Row-blocks are processed r-major (all col-tiles' block r, then r+1)
  so compute starts after only the first DMA pieces land; the PE is
  otherwise idle ~18us waiting on the HBM-bandwidth-shared loads.
- Each 2048-row psum block's exp+column-sum is split across engines:
  ACT: exact exp activation with fused accum_out on [0:ACTW].
  DVE: Schraudolph bitcast exp on [ACTW:] -- int16(A*raw + B0)
  reinterpreted as bf16 IS ~exp(l) (rel err ~3%, zero-mean-tuned;
  S_j err ~0.2%), then a 2-byte tensor_scalar reduce accumulates.
Host: numer_j from exact f64 logits per class block + chip log S_j.
"""
import numpy as np
import ml_dtypes
from contextlib import ExitStack

import concourse.bacc as bacc
import concourse.tile as tile
from concourse import mybir
from concourse.bass_utils import run_bass_kernel_spmd

B, D = 8192, 128
NCLS = 100
NCORES = 8
COLS = B // NCORES            # 1024 columns per core
NCT = COLS // 128             # 8 col-tiles per core
NRB = 4                       # row blocks per col-tile
BLK = B // NRB                # 2048 rows per block
REST = B - BLK                # rows outside the masked window
NSLOT = NCT * NRB * 2         # 2 S-partial slots per block (ACT+DVE)
ACTW = 1536                   # ACT exp width per 2048 block; DVE gets rest
MMW = 512                     # matmul output width (one psum bank)

TEMP = 0.07
EPS = 0.25
SCALE = float(np.float32(1.0) / np.float32(TEMP))
MASKRAW = 4.5                                   # fp8-exact mask magnitude
DELTA = 0.055                                   # Schraudolph bias centering
A8 = float(np.float32(128.0 * np.log2(np.e) * SCALE))
B8 = float(np.float32(128.0 * (127.0 - DELTA)))

_cache = {}


def _build():
    dt = mybir.dt
    nc = bacc.Bacc("TRN2", target_bir_lowering=False, debug=False,
                   num_devices=NCORES)
    # win: this core's masked window rows; plane 0 embeds^T, plane 1 onehot
    etoh_win = nc.dram_tensor("etoh_win", [D, 2 * BLK], dt.float8e4,
                              kind="ExternalInput").ap()
    et_rest = nc.dram_tensor("et_rest", [D, REST], dt.float8e4,
                             kind="ExternalInput").ap()
    # own columns; plane 0 embeds^T, plane 1 -4.5*onehot
    etoh_own = nc.dram_tensor("etoh_own", [D, 2 * COLS], dt.float8e4,
                              kind="ExternalInput").ap()
    out = nc.dram_tensor("out", [128, NSLOT], dt.float32,
                         kind="ExternalOutput").ap()

    with tile.TileContext(nc) as tc:
        with ExitStack() as ctx:
            const_pool = ctx.enter_context(tc.tile_pool(name="consts", bufs=1))
            d_pool = ctx.enter_context(tc.tile_pool(name="dump", bufs=2))
            q_pool = ctx.enter_context(tc.tile_pool(name="qbuf", bufs=2))
            ps_pool = ctx.enter_context(
                tc.tile_pool(name="psum", bufs=2, space="PSUM"))

            # DMA order mirrors first consumption: own columns, then the
            # window in 512-col pieces (both planes per piece) so the first
            # DoubleRow matmul starts after ~0.4 MB, then the rest chunks.
            t_own = const_pool.tile([D, 2, COLS], dt.float8e4)
            for t in range(2):
                nc.sync.dma_start(t_own[:, t, 0:128], etoh_own[:, t * COLS:
                                                               t * COLS + 128])
            t_win = const_pool.tile([D, 2, BLK], dt.float8e4)
            for k in range(BLK // MMW):
                for t in range(2):
                    nc.sync.dma_start(
                        t_win[:, t, k * MMW:(k + 1) * MMW],
                        etoh_win[:, t * BLK + k * MMW:t * BLK + (k + 1) * MMW])
            for t in range(2):
                nc.sync.dma_start(
                    t_own[:, t, 128:], etoh_own[:, t * COLS + 128:
                                                (t + 1) * COLS])
            t_rest = const_pool.tile([D, REST], dt.float8e4)
            for r in range(1, NRB):
                lo = (r - 1) * BLK
                nc.gpsimd.dma_start(t_rest[:, lo:lo + BLK],
                                    et_rest[:, lo:lo + BLK])

            s_part = const_pool.tile([128, NSLOT], dt.float32)
            dummy = const_pool.tile([128, BLK - ACTW], dt.bfloat16)

            # r-major: all col-tiles' window blocks first (they only need
            # t_own + t_win), then the rest blocks chunk by chunk.
            for r in range(NRB):
                for ct in range(NCT):
                    lhs2 = t_own[:, :, ct * 128:(ct + 1) * 128]
                    lhs1 = t_own[:, 0, ct * 128:(ct + 1) * 128]
                    ps = ps_pool.tile([128, BLK], dt.float32, tag="ps")
                    for k in range(BLK // MMW):
                        sl = ps[:, k * MMW:(k + 1) * MMW]
                        if r == 0:
                            nc.tensor.matmul(
                                sl, lhs2, t_win[:, :, k * MMW:(k + 1) * MMW],
                                start=True, stop=True,
                                perf_mode=mybir.MatmulPerfMode.DoubleRow)
                        else:
                            lo = (r - 1) * BLK + k * MMW
                            nc.tensor.matmul(
                                sl, lhs1, t_rest[:, lo:lo + MMW],
                                start=True, stop=True)
                    # split each block's exp: ACT takes [0:ACTW] with
                    # fused accum, DVE Schraudolphs the tail.
                    slot = (ct * NRB + r) * 2
                    dmp = d_pool.tile([128, ACTW], dt.bfloat16, tag="dmp")
                    nc.scalar.activation(
                        dmp[:], ps[:, 0:ACTW],
                        mybir.ActivationFunctionType.Exp, scale=SCALE)
                    q = q_pool.tile([128, BLK - ACTW], dt.int16, tag="q")
                    nc.vector.tensor_scalar(
                        q[:], ps[:, ACTW:], A8, B8,
                        mybir.AluOpType.mult, mybir.AluOpType.add)
                    nc.vector.tensor_scalar(
                        dummy[:], q[:].bitcast(dt.bfloat16), 1.0, None,
                        mybir.AluOpType.mult, mybir.AluOpType.add,
                        accum_out=s_part[:, slot + 1:slot + 2])
                    # ACT's partial sum via an idle-DVE 2-byte reduce
                    dummy2 = d_pool.tile([128, ACTW], dt.bfloat16, tag="dm2")
                    nc.vector.tensor_scalar(
                        dummy2[:], dmp[:], 1.0, None,
                        mybir.AluOpType.mult, mybir.AluOpType.add,
                        accum_out=s_part[:, slot:slot + 1])

            nc.sync.dma_start(out[:], s_part[:])
    nc.compile()
    return nc


def _get_nc():
    if "nc" not in _cache:
        _cache["nc"] = _build()
    return _cache["nc"]


def _prepare(embeds, labels):
    embeds = np.ascontiguousarray(np.asarray(embeds, dtype=np.float32))
    labels_i = np.asarray(labels).astype(np.int64)
    assert embeds.shape == (B, D)

    perm = np.argsort(labels_i, kind="stable")
    lab = labels_i[perm]
    emb = embeds[perm]

    e8 = emb.astype(ml_dtypes.float8_e4m3)           # [B, D]
    et = np.ascontiguousarray(e8.T)                  # [D, B]

    starts = np.searchsorted(lab, np.arange(NCLS), side="left")
    ends = np.searchsorted(lab, np.arange(NCLS), side="right")

    in_maps = []
    for c in range(NCORES):
        lo, hi = c * COLS, (c + 1) * COLS
        r_lo = int(starts[lab[lo]])
        r_hi = int(ends[lab[hi - 1]])
        span = r_hi - r_lo
        assert span <= BLK, f"window overflow: {span}"
        fill = BLK - span
        after = np.arange(r_hi, min(B, r_hi + fill))
        need = fill - len(after)
        before = np.arange(r_lo - need, r_lo) if need > 0 else np.arange(0)
        win_rows = np.concatenate([np.arange(r_lo, r_hi), after, before])
        assert len(win_rows) == BLK
        rest_mask = np.ones(B, dtype=bool)
        rest_mask[win_rows] = False
        rest_idx = np.nonzero(rest_mask)[0]

        etoh_win = np.zeros((D, 2, BLK), dtype=ml_dtypes.float8_e4m3)
        etoh_win[:, 0, :] = et[:, win_rows]
        oh = np.zeros((D, BLK), dtype=np.float32)
        oh[lab[win_rows], np.arange(BLK)] = 1.0
        etoh_win[:, 1, :] = oh.astype(ml_dtypes.float8_e4m3)

        etoh_own = np.zeros((D, 2, COLS), dtype=ml_dtypes.float8_e4m3)
        etoh_own[:, 0, :] = et[:, lo:hi]
        ohn = np.zeros((D, COLS), dtype=np.float32)
        ohn[lab[lo:hi], np.arange(COLS)] = -MASKRAW
        etoh_own[:, 1, :] = ohn.astype(ml_dtypes.float8_e4m3)

        in_maps.append({
            "etoh_win": np.ascontiguousarray(etoh_win.reshape(D, 2 * BLK)),
            "et_rest": np.ascontiguousarray(et[:, rest_idx]),
            "etoh_own": np.ascontiguousarray(etoh_own.reshape(D, 2 * COLS)),
        })
    return in_maps, lab, emb


def _combine(results, lab, emb):
    S = np.empty(B, dtype=np.float64)
    for c in range(NCORES):
        o = results[c]["out"].astype(np.float64)     # [128, NSLOT]
        s = o.reshape(128, NCT, NRB * 2).sum(-1)     # [p, ct]
        S[c * COLS:(c + 1) * COLS] = s.T.reshape(-1)  # j = ct*128 + p

    counts = np.bincount(lab, minlength=NCLS)
    embf = emb.astype(np.float64)
    logS = np.log(S)
    loss = 0.0
    for cl in range(NCLS):
        idx = np.nonzero(lab == cl)[0]
        n = len(idx)
        if n < 2:
            continue
        Ec = embf[idx]
        Lc = (Ec @ Ec.T) / TEMP
        x = np.exp(Lc - EPS) / S[idx][None, :]
        ce = logS[idx][None, :] - Lc + np.log1p(x)
        np.fill_diagonal(ce, 0.0)
        loss += (ce.sum(axis=0) / (n - 1.0)).sum()
    loss /= B
    return np.asarray(loss, dtype=np.float32)


def kernel(embeds, labels):
    in_maps, lab, emb = _prepare(embeds, labels)
    nc = _get_nc()
    res = run_bass_kernel_spmd(nc, in_maps, list(range(NCORES)))
    return _combine(res.results, lab, emb)
